# revision 1
# baseline (speedup 1.0000x reference)
"""AttnBlock2D Trainium2 kernel (8-core data-parallel over batch), fp8 edition.

Per core: one batch element. x:[512, 4096] (c, h*w).
  h = GroupNorm32(x) * scale + bias            (f32 stats, h stored fp8e4m3)
  q = wq@h, k = wk@h, v = wv@h                 (fp8 DoubleRow matmuls)
  attn = softmax(q^T k / sqrt(512));  out = v @ attn^T   (fp8 DoubleRow)
  y = x + wp@out + bp                          (residual in f32)

All heavy matmuls use fp8e4m3 operands with MatmulPerfMode.DoubleRow:
contraction of 256 (2x128 channel/key pairs packed along the free dim) per
pass at ~2x the f32r MAC rate (measured 146ns vs 265ns for the same MACs).
The rel-err budget (2e-2) dwarfs fp8 noise: the attention branch enters the
output through a 1x1 conv whose magnitude is ~40x below the residual x.

Everything is SBUF-resident (q,k,v,h fp8 = 2MB each) - no DRAM staging.
Softmax skips max-subtraction (logits ~N(0,1)); exp is biased by -1 so the
max representable pt stays well under fp8e4m3's 448 ceiling; the bias
cancels in the softmax normalization. The denominator is accumulated on the
PE with a ones-column DoubleRow matmul (no vector-engine tree).

PSUM accumulation-chain convention: a 2KB PSUM bank hosts two 256-wide
half-chains; only the FIRST matmul touching the bank sets start=True (the
pending-zero region is the whole bank, so the other half's first touch
auto-replaces), and only the LAST touching matmul sets stop=True.
"""
import os
import numpy as np
import ml_dtypes

P = 128
C = 512
NCH = C // P              # 4 chunks of 128 channels
NCJ = 2                   # 2 pair-chunks of 256 channels
HW = 4096                 # 64*64 pixels
QB = 512                  # query block
NQB = HW // QB            # 8
NE = HW // P              # 32 key chunks
NE2 = NE // 2             # 16 key pair-chunks
EPS = 1e-5
SCALE = 1.0 / np.sqrt(C)
EXPB = -2.0               # exp(s*SCALE + EXPB); cancels in normalization
B = 8                     # batch / cores

_CACHE = {}

KNOBS = {
    "pp1": 4,    # phase-B qkv psum bufs
    "ptp": 6,    # pt (exp output) bufs
    "ssp": 4,    # S psum bufs
    "smp": 1,    # dn/rb psum bufs
    "osp": 2,    # normalized-out fp8 bufs (per-cj pairs)
    "fin": 4,
    "tmp": 3,
}


def _emit(nc, tc, ctx):
    import concourse.bass as bass
    from concourse import mybir
    from contextlib import ExitStack

    f32 = mybir.dt.float32
    f32r = mybir.dt.float32r
    f8 = mybir.dt.float8e4
    AF = mybir.ActivationFunctionType
    OP = mybir.AluOpType
    DR = mybir.MatmulPerfMode.DoubleRow

    # ---------------- I/O ----------------
    x_d = nc.declare_dram_parameter("x", [C, HW], f32, isOutput=False).ap()
    wq_d = nc.declare_dram_parameter("wq8", [NCJ * P, 2, C], f8, isOutput=False).ap()
    wk_d = nc.declare_dram_parameter("wk8", [NCJ * P, 2, C], f8, isOutput=False).ap()
    wv_d = nc.declare_dram_parameter("wv8", [NCJ * P, 2, C], f8, isOutput=False).ap()
    wp_d = nc.declare_dram_parameter("wp8", [NCJ * P, 2, C], f8, isOutput=False).ap()
    bq_d = nc.declare_dram_parameter("bq", [C], f32, isOutput=False).ap()
    bk_d = nc.declare_dram_parameter("bk", [C], f32, isOutput=False).ap()
    bv_d = nc.declare_dram_parameter("bv", [C], f32, isOutput=False).ap()
    bp_d = nc.declare_dram_parameter("bp", [C], f32, isOutput=False).ap()
    ns_d = nc.declare_dram_parameter("nscale", [C], f32, isOutput=False).ap()
    nb_d = nc.declare_dram_parameter("nbias", [C], f32, isOutput=False).ap()
    i16_d = nc.declare_dram_parameter("ind16", [P, 8], f32, isOutput=False).ap()
    iT_d = nc.declare_dram_parameter("indT", [8, P], f32, isOutput=False).ap()
    oc_d = nc.declare_dram_parameter("ones_col", [P, 1], f32r, isOutput=False).ap()
    or_d = nc.declare_dram_parameter("ones_row", [1, P], f32r, isOutput=False).ap()
    out_d = nc.declare_dram_parameter("out", [C, HW], f32, isOutput=True).ap()

    def col_ap(src, ci):
        # [128] slice of a [512] DRAM vector viewed as [128, 1]
        return bass.AP(tensor=src.tensor, offset=ci * P, ap=[[1, P], [0, 1]])

    # ---------------- persistent pools ----------------
    cst = ctx.enter_context(tc.tile_pool(name="cst", bufs=1))
    wq_sb, wk_sb, wv_sb, wp_sb = [], [], [], []
    for nm, dst, srcd in (("wq", wq_sb, wq_d), ("wk", wk_sb, wk_d),
                          ("wv", wv_sb, wv_d), ("wp", wp_sb, wp_d)):
        for cj in range(NCJ):
            t = cst.tile([P, 2, C], f8, name=f"{nm}{cj}", tag=f"{nm}{cj}")
            nc.sync.dma_start(out=t, in_=srcd[cj * P:(cj + 1) * P])
            dst.append(t)
    bq_sb, bk_sb, bp_sb = [], [], []
    for m in range(NCH):
        t = cst.tile([P, 1], f32, name=f"bq{m}", tag=f"bq{m}")
        nc.sync.dma_start(out=t, in_=col_ap(bq_d, m))
        bq_sb.append(t)
        t = cst.tile([P, 1], f32, name=f"bk{m}", tag=f"bk{m}")
        nc.sync.dma_start(out=t, in_=col_ap(bk_d, m))
        bk_sb.append(t)
        t = cst.tile([P, 1], f32, name=f"bp{m}", tag=f"bp{m}")
        nc.sync.dma_start(out=t, in_=col_ap(bp_d, m))
        bp_sb.append(t)
    bv_raw = cst.tile([1, C], f32, name="bv_raw", tag="bv_raw")
    nc.sync.dma_start(out=bv_raw,
                      in_=bass.AP(tensor=bv_d.tensor, offset=0,
                                  ap=[[0, 1], [1, C]]))
    bv_row = cst.tile([1, C], f32r, name="bv_row", tag="bv_row")
    nc.vector.tensor_copy(out=bv_row, in_=bv_raw)
    ind16_sb = cst.tile([P, 8], f32, name="ind16", tag="ind16")
    nc.sync.dma_start(out=ind16_sb, in_=i16_d)
    indT_sb = cst.tile([8, P], f32, name="indT", tag="indT")
    nc.sync.dma_start(out=indT_sb, in_=iT_d)
    onec_sb = cst.tile([P, 1], f32r, name="onec", tag="onec")
    nc.sync.dma_start(out=onec_sb, in_=oc_d)
    oner_sb = cst.tile([1, P], f32r, name="oner", tag="oner")
    nc.sync.dma_start(out=oner_sb, in_=or_d)
    expb_sb = cst.tile([P, 1], f32, name="expb", tag="expb")
    nc.vector.memset(expb_sb, EXPB)
    osc_sb = cst.tile([P, 1], f32, name="osc", tag="osc")
    nc.vector.memset(osc_sb, 0.125)

    x_pool = ctx.enter_context(tc.tile_pool(name="xres", bufs=1, side="right"))
    x_sb = [x_pool.tile([P, HW], f32, name=f"x{m}", tag=f"x{m}")
            for m in range(NCH)]
    for m in range(NCH):
        for hf in range(2):
            nc.sync.dma_start(out=x_sb[m][:, hf * 2048:(hf + 1) * 2048],
                              in_=x_d[m * P:(m + 1) * P,
                                      hf * 2048:(hf + 1) * 2048])

    qk_pool = ctx.enter_context(tc.tile_pool(name="qkres", bufs=1))
    q_sb = [qk_pool.tile([P, 2, HW], f8, name=f"q{cj}", tag=f"q{cj}")
            for cj in range(NCJ)]
    k_sb = [qk_pool.tile([P, 2, HW], f8, name=f"k{cj}", tag=f"k{cj}")
            for cj in range(NCJ)]
    vt_pool = ctx.enter_context(tc.tile_pool(name="vtres", bufs=1))
    vt_sb = [vt_pool.tile([P, 2, C], f8, name=f"vt{e2}", tag=f"vt{e2}")
             for e2 in range(NE2)]

    repeat = int(os.environ.get("ATTN_REPEAT", "1"))
    for rep in range(repeat):
        _emit_body(nc, tc, rep, locals())


def _emit_body(nc, tc, rep, env):
    import concourse.bass as bass
    from concourse import mybir
    from contextlib import ExitStack

    f32 = mybir.dt.float32
    f32r = mybir.dt.float32r
    f8 = mybir.dt.float8e4
    AF = mybir.ActivationFunctionType
    OP = mybir.AluOpType
    DR = mybir.MatmulPerfMode.DoubleRow

    (x_sb, q_sb, k_sb, vt_sb, wq_sb, wk_sb, wv_sb, wp_sb, bq_sb, bk_sb,
     bp_sb, bv_row, ind16_sb, indT_sb, onec_sb, oner_sb, expb_sb, osc_sb,
     ns_d, nb_d, out_d, col_ap) = (
        env["x_sb"], env["q_sb"], env["k_sb"], env["vt_sb"], env["wq_sb"],
        env["wk_sb"], env["wv_sb"], env["wp_sb"], env["bq_sb"], env["bk_sb"],
        env["bp_sb"], env["bv_row"], env["ind16_sb"], env["indT_sb"],
        env["onec_sb"], env["oner_sb"], env["expb_sb"], env["osc_sb"],
        env["ns_d"], env["nb_d"], env["out_d"], env["col_ap"])

    h_pool = tc.alloc_tile_pool(name=f"hres{rep}", bufs=1, side="right")
    h_sb = [h_pool.tile([P, 2, HW], f8, name=f"h{cj}", tag=f"h{cj}")
            for cj in range(NCJ)]

    # ================ phase A: groupnorm ================
    with ExitStack() as s1:
        gn = s1.enter_context(tc.tile_pool(name="gn", bufs=2))
        gnp = s1.enter_context(tc.tile_pool(name="gnp", bufs=2, space="PSUM"))
        nsc_sb, nbs_sb = [], []
        for ci in range(NCH):
            t = gn.tile([P, 1], f32, name=f"nsc{ci}", tag=f"nsc{ci}", bufs=1)
            nc.sync.dma_start(out=t, in_=col_ap(ns_d, ci))
            nsc_sb.append(t)
            t = gn.tile([P, 1], f32, name=f"nbs{ci}", tag=f"nbs{ci}", bufs=1)
            nc.sync.dma_start(out=t, in_=col_ap(nb_d, ci))
            nbs_sb.append(t)

        eps_t = gn.tile([8, 1], f32, name="eps_t", tag="eps", bufs=1)
        nc.vector.memset(eps_t, EPS)
        # fully per-chunk pipeline: groups are chunk-local, so each chunk's
        # aggregation + h-write runs as soon as ITS stats land, overlapping
        # later chunks' bn_stats instead of waiting on a cross-chunk barrier.
        for ci in range(NCH):
            stats = gn.tile([P, 8, 6], f32, name=f"st{ci}", tag="st")
            for s in range(8):
                nc.vector.bn_stats(out=stats[:, s, :],
                                   in_=x_sb[ci][:, s * 512:(s + 1) * 512])
            mv = gn.tile([P, 2], f32, name=f"mv{ci}", tag="mv")
            nc.vector.bn_aggr(out=mv, in_=stats)
            m2 = gn.tile([P, 2], f32, name=f"m2{ci}", tag="m2")
            nc.vector.tensor_copy(out=m2[:, 0:1], in_=mv[:, 0:1])
            msq = gn.tile([P, 1], f32, name=f"msq{ci}", tag="msq")
            nc.vector.tensor_mul(out=msq, in0=mv[:, 0:1], in1=mv[:, 0:1])
            nc.vector.tensor_add(out=m2[:, 1:2], in0=mv[:, 1:2], in1=msq)

            g_ps = gnp.tile([8, 2], f32, name=f"g{ci}", tag="gps")
            nc.tensor.matmul(g_ps, lhsT=ind16_sb, rhs=m2, start=True,
                             stop=True)
            g_sb = gn.tile([8, 2], f32, name=f"gsb{ci}", tag="gsb")
            nc.vector.tensor_copy(out=g_sb, in_=g_ps)
            msq2 = gn.tile([8, 1], f32, name=f"ms2{ci}", tag="msq2")
            nc.vector.tensor_mul(out=msq2, in0=g_sb[:, 0:1], in1=g_sb[:, 0:1])
            var_g = gn.tile([8, 1], f32, name=f"vg{ci}", tag="varg")
            nc.vector.tensor_sub(out=var_g, in0=g_sb[:, 1:2], in1=msq2)
            std_g = gn.tile([8, 1], f32, name=f"sg{ci}", tag="stdg")
            nc.scalar.activation(out=std_g, in_=var_g, func=AF.Sqrt,
                                 bias=eps_t, scale=1.0)
            rstd_g = gn.tile([8, 1], f32, name=f"rg{ci}", tag="rstdg")
            nc.vector.reciprocal(out=rstd_g, in_=std_g)
            rb2 = gn.tile([8, 2], f32, name=f"rb2{ci}", tag="rb2")
            nc.vector.tensor_copy(out=rb2[:, 0:1], in_=rstd_g)
            nc.vector.tensor_mul(out=rb2[:, 1:2], in0=g_sb[:, 0:1],
                                 in1=rstd_g)

            ab_ps = gnp.tile([P, 2], f32, name=f"ab{ci}", tag="gps")
            nc.tensor.matmul(ab_ps, lhsT=indT_sb, rhs=rb2, start=True,
                             stop=True)
            A_t = gn.tile([P, 1], f32, name=f"A{ci}", tag="A")
            nc.vector.tensor_mul(out=A_t, in0=ab_ps[:, 0:1], in1=nsc_sb[ci])
            t0 = gn.tile([P, 1], f32, name=f"t0{ci}", tag="t0")
            nc.vector.tensor_mul(out=t0, in0=ab_ps[:, 1:2], in1=nsc_sb[ci])
            B_t = gn.tile([P, 1], f32, name=f"B{ci}", tag="Bt")
            nc.vector.tensor_sub(out=B_t, in0=nbs_sb[ci], in1=t0)
            eng = nc.vector if ci % 2 == 0 else nc.gpsimd
            for hf in range(2):
                eng.tensor_scalar(
                    out=h_sb[ci // 2][:, ci % 2, hf * 2048:(hf + 1) * 2048],
                    in0=x_sb[ci][:, hf * 2048:(hf + 1) * 2048],
                    scalar1=A_t, scalar2=B_t,
                    op0=OP.mult, op1=OP.add)

    # fold bp into the resident x: the phase-C tail then skips its Act bias
    # step (y = pj*rb + (x + bp)). Must come after the h-writes read x.
    for oc in range(NCH):
        eng = nc.vector if oc % 2 == 0 else nc.gpsimd
        eng.tensor_scalar_add(out=x_sb[oc], in0=x_sb[oc],
                              scalar1=bp_sb[oc])

    # ================ phase B: q, k, vT projections (all SBUF-resident) ======
    with ExitStack() as s2:
        pp1 = s2.enter_context(tc.tile_pool(name="pp1", bufs=KNOBS["pp1"],
                                            space="PSUM"))
        # emitted in pixel-block order, k/q/v interleaved, so phase C's
        # early S/PV chains (which need only low pixel blocks of k/q/vt)
        # can overlap with the tail of phase B.
        for nb in range(NQB):
            for wsb, bsb, dst in ((wk_sb, bk_sb, k_sb), (wq_sb, bq_sb, q_sb)):
                for m in range(NCH):
                    ps = pp1.tile([P, QB], f32, name="qkps", tag="mm")
                    for h in range(2):
                        for cj in range(NCJ):
                            nc.tensor.matmul(
                                ps[:, h * 256:(h + 1) * 256],
                                lhsT=wsb[cj][:, :, m * P:(m + 1) * P],
                                rhs=h_sb[cj][:, :, nb * QB + h * 256:
                                             nb * QB + (h + 1) * 256],
                                start=(h == 0 and cj == 0),
                                stop=(h == 1 and cj == NCJ - 1),
                                perf_mode=DR)
                    if m % 2 == 0:
                        nc.scalar.activation(
                            out=dst[m // 2][:, m % 2, nb * QB:(nb + 1) * QB],
                            in_=ps, func=AF.Identity, bias=bsb[m], scale=1.0)
                    else:
                        nc.vector.tensor_scalar(
                            out=dst[m // 2][:, m % 2, nb * QB:(nb + 1) * QB],
                            in0=ps, scalar1=bsb[m], scalar2=None, op0=OP.add)
            # vT for the 4 key chunks of this pixel block. bv is seeded
            # into the PSUM chain by a broadcast f32r matmul (ones-row x
            # bv-row), so the conversion is a plain copy splittable
            # between DVE and Act.
            for e in range(nb * 4, nb * 4 + 4):
                ps = pp1.tile([P, C], f32, name="vps", tag="mm")
                nc.tensor.matmul(ps, lhsT=oner_sb, rhs=bv_row,
                                 start=True, stop=False)
                for h in range(2):
                    for cj in range(NCJ):
                        nc.tensor.matmul(
                            ps[:, h * 256:(h + 1) * 256],
                            lhsT=h_sb[cj][:, :, e * P:(e + 1) * P],
                            rhs=wv_sb[cj][:, :, h * 256:(h + 1) * 256],
                            start=False,
                            stop=(h == 1 and cj == NCJ - 1),
                            perf_mode=DR)
                if e % 2 == 0:
                    nc.vector.tensor_copy(out=vt_sb[e // 2][:, e % 2, :],
                                          in_=ps)
                else:
                    nc.scalar.activation(out=vt_sb[e // 2][:, e % 2, :],
                                         in_=ps, func=AF.Copy,
                                         bias=0.0, scale=1.0)

    h_pool.release()

    # ================ phase C: attention + proj ================
    with ExitStack() as s3:
        ptp = s3.enter_context(tc.tile_pool(name="ptp", bufs=KNOBS["ptp"]))
        smp = s3.enter_context(tc.tile_pool(name="smp", bufs=2))
        osp = s3.enter_context(tc.tile_pool(name="osp", bufs=KNOBS["osp"]))
        fnp = s3.enter_context(tc.tile_pool(name="fnp", bufs=4))
        pv_ps_pool = s3.enter_context(tc.tile_pool(name="pvp", bufs=4,
                                                   space="PSUM"))
        s_ps_pool = s3.enter_context(tc.tile_pool(name="ssp", bufs=KNOBS["ssp"],
                                                  space="PSUM"))

        for qb in range(NQB):
            pvs = [pv_ps_pool.tile([P, QB], f32, name=f"pv{co}", tag="pv")
                   for co in range(NCH)]
            acc0 = smp.tile([P, QB], f32r, name="acc0", tag="acc0")
            acc1 = smp.tile([P, QB], f32r, name="acc1", tag="acc1")
            for e2 in range(NE2):
                pt = ptp.tile([P, 2, QB], f8, name="pt", tag="pt")
                for sub in range(2):
                    e = 2 * e2 + sub
                    s_ps = s_ps_pool.tile([P, QB], f32, name="s_ps", tag="s")
                    for h in range(2):
                        for cj in range(NCJ):
                            nc.tensor.matmul(
                                s_ps[:, h * 256:(h + 1) * 256],
                                lhsT=k_sb[cj][:, :, e * P:(e + 1) * P],
                                rhs=q_sb[cj][:, :, qb * QB + h * 256:
                                             qb * QB + (h + 1) * 256],
                                start=(h == 0 and cj == 0),
                                stop=(h == 1 and cj == NCJ - 1),
                                perf_mode=DR)
                    nc.scalar.activation(out=pt[:, sub, :], in_=s_ps,
                                         func=AF.Exp, bias=expb_sb,
                                         scale=float(SCALE))
                # denominator: sub=0 chain on DVE, sub=1 chain on Pool
                if e2 == 0:
                    nc.vector.tensor_copy(out=acc0, in_=pt[:, 0, :])
                    nc.gpsimd.tensor_copy(out=acc1, in_=pt[:, 1, :])
                else:
                    nc.vector.tensor_add(out=acc0, in0=acc0, in1=pt[:, 0, :])
                    nc.gpsimd.tensor_add(out=acc1, in0=acc1, in1=pt[:, 1, :])
                for co in range(NCH):
                    for h in range(2):
                        nc.tensor.matmul(
                            pvs[co][:, h * 256:(h + 1) * 256],
                            lhsT=vt_sb[e2][:, :, co * P:(co + 1) * P],
                            rhs=pt[:, :, h * 256:(h + 1) * 256],
                            start=(e2 == 0 and h == 0),
                            stop=(e2 == NE2 - 1 and h == 1),
                            perf_mode=DR)
            # unnormalized PV out * 1/8 -> fp8 (IEEE e4m3 saturates at 240;
            # raw pvs reaches ~300). The 8x folds into rb via the dn copy
            # scale. Normalization commutes with the 1x1 conv: rb varies
            # only along queries, proj contracts channels. Emitted FIRST and
            # split DVE/Pool so the pvs banks free quickly for the next qb.
            outp = [osp.tile([P, 2, QB], f8, name=f"op{cj}", tag="osb")
                    for cj in range(NCJ)]
            for m in range(NCH):
                if m % 2 == 0:
                    nc.vector.tensor_scalar(out=outp[m // 2][:, m % 2, :],
                                            in0=pvs[m], scalar1=osc_sb,
                                            scalar2=None, op0=OP.mult)
                else:
                    nc.scalar.activation(out=outp[m // 2][:, m % 2, :],
                                         in_=pvs[m], func=AF.Copy,
                                         bias=0.0, scale=0.125)
            dn_ps = s_ps_pool.tile([1, QB], f32, name="dn_ps", tag="s")
            nc.tensor.matmul(dn_ps, lhsT=onec_sb, rhs=acc0, start=True,
                             stop=False)
            nc.tensor.matmul(dn_ps, lhsT=onec_sb, rhs=acc1, start=False,
                             stop=True)
            dn_sb = smp.tile([1, QB], f32r, name="dn_sb", tag="dnsb", bufs=1)
            nc.scalar.activation(out=dn_sb, in_=dn_ps, func=AF.Copy,
                                 bias=0.0, scale=0.125)
            rb_ps = s_ps_pool.tile([P, QB], f32, name="rb_ps", tag="s")
            nc.tensor.matmul(rb_ps, lhsT=oner_sb, rhs=dn_sb, start=True,
                             stop=True)
            rb_sb = smp.tile([P, QB], f32, name="rb_sb", tag="rbsb")
            nc.vector.reciprocal(out=rb_sb, in_=rb_ps)
            for oc in range(NCH):
                pj_ps = pv_ps_pool.tile([P, QB], f32, name="pj_ps", tag="pv")
                for h in range(2):
                    for cj in range(NCJ):
                        nc.tensor.matmul(
                            pj_ps[:, h * 256:(h + 1) * 256],
                            lhsT=wp_sb[cj][:, :, oc * P:(oc + 1) * P],
                            rhs=outp[cj][:, :, h * 256:(h + 1) * 256],
                            start=(h == 0 and cj == 0),
                            stop=(h == 1 and cj == NCJ - 1),
                            perf_mode=DR)
                t_n = fnp.tile([P, QB], f32, name="t_n", tag="tn",
                               bufs=KNOBS["tmp"])
                nc.vector.tensor_mul(out=t_n, in0=pj_ps, in1=rb_sb)
                fin = fnp.tile([P, QB], f32, name="fin", tag="fin",
                               bufs=KNOBS["fin"])
                eng = nc.gpsimd if oc % 2 == 0 else nc.vector
                eng.tensor_add(out=fin, in0=t_n,
                               in1=x_sb[oc][:, qb * QB:(qb + 1) * QB])
                nc.sync.dma_start(out=out_d[oc * P:(oc + 1) * P,
                                            qb * QB:(qb + 1) * QB], in_=fin)


def build_nc():
    import concourse.bacc as bacc
    import concourse.tile as tile
    from contextlib import ExitStack

    nc = bacc.Bacc("TRN2", target_bir_lowering=False, debug=False)
    with tile.TileContext(nc) as tc:
        with ExitStack() as ctx:
            _emit(nc, tc, ctx)
    nc.finalize()
    return nc


def host_constants():
    ind16 = np.zeros((P, 8), np.float32)
    for p in range(P):
        ind16[p, p // 16] = 1.0 / 16.0
    indT = np.zeros((8, P), np.float32)
    for p in range(P):
        indT[p // 16, p] = 1.0
    ones_col = np.ones((P, 1), np.float32)
    ones_row = np.ones((1, P), np.float32)
    return ind16, indT, ones_col, ones_row


def _pack_wT(w):
    # w: [c_out, c_in] f32.  Return [cj*128+p, j, c_out] fp8 where
    # c_in = cj*256 + j*128 + p.
    wT = np.ascontiguousarray(np.asarray(w, np.float32).T)      # [c_in, c_out]
    wT = wT.reshape(NCJ, 2, P, C).transpose(0, 2, 1, 3)          # [cj, p, j, o]
    return np.ascontiguousarray(wT.reshape(NCJ * P, 2, C)).astype(
        ml_dtypes.float8_e4m3)


def make_in_maps(inputs):
    x = np.asarray(inputs["x"], np.float32)
    ind16, indT, ones_col, ones_row = host_constants()
    shared = {
        "wq8": _pack_wT(inputs["wq"]),
        "wk8": _pack_wT(inputs["wk"]),
        "wv8": _pack_wT(inputs["wv"]),
        "wp8": _pack_wT(inputs["wp"]),
        "bq": np.asarray(inputs["bq"], np.float32),
        "bk": np.asarray(inputs["bk"], np.float32),
        "bv": np.asarray(inputs["bv"], np.float32),
        "bp": np.asarray(inputs["bp"], np.float32),
        "nscale": np.asarray(inputs["norm_scale"], np.float32),
        "nbias": np.asarray(inputs["norm_bias"], np.float32),
        "ind16": ind16, "indT": indT,
        "ones_col": ones_col, "ones_row": ones_row,
    }
    return [dict(shared, x=np.ascontiguousarray(x[i].reshape(C, HW)))
            for i in range(B)]


def kernel(**inputs):
    from concourse.bass_utils import run_bass_kernel_spmd

    if "nc" not in _CACHE:
        _CACHE["nc"] = build_nc()
    nc = _CACHE["nc"]
    in_maps = make_in_maps(inputs)
    res = run_bass_kernel_spmd(nc, in_maps, list(range(B)))
    out = np.stack([res.results[i]["out"] for i in range(B)])
    return out.reshape(B, C, 64, 64)



# revision 5
# speedup vs baseline: 1.1402x; 1.1402x over previous
"""AttnBlock2D Trainium2 kernel (8-core data-parallel over batch), fp8 edition.

Per core: one batch element. x:[512, 4096] (c, h*w).
  h = GroupNorm32(x) * scale + bias            (f32 stats, h stored fp8e4m3)
  q = wq@h, k = wk@h, v = wv@h                 (fp8 DoubleRow matmuls)
  attn = softmax(q^T k / sqrt(512));  out = v @ attn^T   (fp8 DoubleRow)
  y = x + wp@out + bp                          (residual in f32)

All heavy matmuls use fp8e4m3 operands with MatmulPerfMode.DoubleRow at
FD=512 (full PSUM bank per matmul; rhs streams [128,2,512] = the 1024-elem
moving-operand max), halving instruction count and LDWEIGHTS traffic vs a
256-wide split. The rel-err budget (2e-2) dwarfs fp8 noise: the attention
branch enters the output through a 1x1 conv whose magnitude is ~40x below
the residual x.

Softmax skips max-subtraction (logits ~N(0,1)); exp is biased by -2 so the
max representable stays well under fp8e4m3's 448 ceiling; the bias cancels
in the normalization. The denominator is accumulated on the PE with an
fp8-ones DoubleRow chain over the exp tiles (pt), not on DVE/gpsimd --
fp8-source adds on those engines measure ~1.3us/[128,512] and were the
phase-C near-critical engines in the 635us baseline.

DMA: all small constants ship as one packed [128,32] f32 tensor and the 4
weight matrices as one [1024,2,512] fp8 tensor; x goes first, split across
BOTH hardware DGE queues (sync + scalar). The 635us baseline pushed 9.4MB
through one queue behind ~3300 4-byte packets of per-column bias loads.

gpsimd runs only the phase-C residual adds: in-place [128,4096]
tensor_scalar on gpsimd measures 58us (!) and stalls concurrent DVE ops,
which was the root cause of the baseline's 63us A->B and 39us B->C
pipeline gaps.

PSUM budget per phase-C qb: 4 banks PV accumulators + 3 S-score banks + 1
denominator/rebroadcast bank = 8.
"""
import os
import numpy as np
import ml_dtypes

P = 128
C = 512
NCH = C // P              # 4 chunks of 128 channels
NCJ = 2                   # 2 pair-chunks of 256 channels
HW = 4096                 # 64*64 pixels
QB = 512                  # query block
NQB = HW // QB            # 8
NE = HW // P              # 32 key chunks
NE2 = NE // 2             # 16 key pair-chunks
EPS = 1e-5
SCALE = 1.0 / np.sqrt(C)
EXPB = -2.0               # exp(s*SCALE + EXPB); cancels in normalization
B = 8                     # batch / cores

_CACHE = {}

KNOBS = {
    "pp1": 4,    # phase-B qkv psum bufs
    "ptp": 6,    # pt (exp output) bufs
    "ssp": 3,    # S psum bufs
    "osp": 2,    # normalized-out fp8 bufs (per-cj pairs)
    "fin": 4,
    "tmp": 3,
}

# consts[128, 32] column layout
CB_Q, CB_K, CB_P, CB_NS, CB_NB, CB_I16 = 0, 4, 8, 12, 16, 20


def _emit(nc, tc, ctx):
    import concourse.bass as bass
    from concourse import mybir
    from contextlib import ExitStack

    f32 = mybir.dt.float32
    f32r = mybir.dt.float32r
    f8 = mybir.dt.float8e4

    # ---------------- I/O ----------------
    x_d = nc.declare_dram_parameter("x", [C, HW], f32, isOutput=False).ap()
    w8_d = nc.declare_dram_parameter("w8", [4 * NCJ * P, 2, C], f8,
                                     isOutput=False).ap()
    cst_d = nc.declare_dram_parameter("consts", [P, 32], f32,
                                      isOutput=False).ap()
    iT_d = nc.declare_dram_parameter("indT", [8, P], f32, isOutput=False).ap()
    bv_d = nc.declare_dram_parameter("bv", [C], f32, isOutput=False).ap()
    out_d = nc.declare_dram_parameter("out", [C, HW], f32, isOutput=True).ap()

    # ---------------- persistent pools ----------------
    # x residual first: it is the latency-critical DMA. Alternate the two
    # hardware DGE queues (sync + scalar) so the 8MB load runs dual-stream.
    x_pool = ctx.enter_context(tc.tile_pool(name="xres", bufs=1, side="right"))
    x_sb = [x_pool.tile([P, HW], f32, name=f"x{m}", tag=f"x{m}")
            for m in range(NCH)]
    qi = 0
    for m in range(NCH):
        for hf in range(2):
            eng = nc.sync if qi % 2 == 0 else nc.scalar
            qi += 1
            eng.dma_start(out=x_sb[m][:, hf * 2048:(hf + 1) * 2048],
                          in_=x_d[m * P:(m + 1) * P,
                                  hf * 2048:(hf + 1) * 2048])

    cst = ctx.enter_context(tc.tile_pool(name="cst", bufs=1))
    cst_sb = cst.tile([P, 32], f32, name="consts", tag="consts")
    nc.sync.dma_start(out=cst_sb, in_=cst_d)
    indT_sb = cst.tile([8, P], f32, name="indT", tag="indT")
    nc.sync.dma_start(out=indT_sb, in_=iT_d)
    bv_raw = cst.tile([1, C], f32, name="bv_raw", tag="bv_raw")
    nc.sync.dma_start(out=bv_raw,
                      in_=bass.AP(tensor=bv_d.tensor, offset=0,
                                  ap=[[0, 1], [1, C]]))
    bv_row = cst.tile([1, C], f32r, name="bv_row", tag="bv_row")
    nc.vector.tensor_copy(out=bv_row, in_=bv_raw)

    bq_sb = [cst_sb[:, CB_Q + m:CB_Q + m + 1] for m in range(NCH)]
    bk_sb = [cst_sb[:, CB_K + m:CB_K + m + 1] for m in range(NCH)]
    bp_sb = [cst_sb[:, CB_P + m:CB_P + m + 1] for m in range(NCH)]
    nsc_sb = [cst_sb[:, CB_NS + m:CB_NS + m + 1] for m in range(NCH)]
    nbs_sb = [cst_sb[:, CB_NB + m:CB_NB + m + 1] for m in range(NCH)]
    ind16_sb = cst_sb[:, CB_I16:CB_I16 + 8]

    # f32r / fp8 memset is invalid ISA -- memset f32 then cast-copy
    ones8f = cst.tile([P, 32], f32, name="ones8f", tag="ones8f")
    nc.vector.memset(ones8f, 1.0)
    oner_f = cst.tile([1, P], f32, name="oner_f", tag="oner_f")
    nc.vector.memset(oner_f, 1.0)
    oner_sb = cst.tile([1, P], f32r, name="oner", tag="oner")
    nc.vector.tensor_copy(out=oner_sb, in_=oner_f)
    # fp8 ones for the PE denominator chain; pair-dim stride padded to 16B
    ones8 = cst.tile([P, 2, 16], f8, name="ones8", tag="ones8")
    nc.vector.tensor_copy(out=ones8, in_=ones8f)
    expb_sb = cst.tile([P, 1], f32, name="expb", tag="expb")
    nc.vector.memset(expb_sb, EXPB)
    osc_sb = cst.tile([P, 1], f32, name="osc", tag="osc")
    nc.vector.memset(osc_sb, 0.125)

    # weights: emitted after x so x wins the queue-head; k first (first used)
    wq_sb, wk_sb, wv_sb, wp_sb = [], [], [], []
    qi = 0
    for nm, dst, base in (("wk", wk_sb, 1), ("wq", wq_sb, 0),
                          ("wv", wv_sb, 2), ("wp", wp_sb, 3)):
        for cj in range(NCJ):
            t = cst.tile([P, 2, C], f8, name=f"{nm}{cj}", tag=f"{nm}{cj}")
            off = base * NCJ * P + cj * P
            eng = nc.sync if qi % 2 == 0 else nc.scalar
            qi += 1
            eng.dma_start(out=t, in_=w8_d[off:off + P])
            dst.append(t)

    qk_pool = ctx.enter_context(tc.tile_pool(name="qkres", bufs=1))
    q_sb = [qk_pool.tile([P, 2, HW], f8, name=f"q{cj}", tag=f"q{cj}")
            for cj in range(NCJ)]
    k_sb = [qk_pool.tile([P, 2, HW], f8, name=f"k{cj}", tag=f"k{cj}")
            for cj in range(NCJ)]
    vt_pool = ctx.enter_context(tc.tile_pool(name="vtres", bufs=1))
    vt_sb = [vt_pool.tile([P, 2, C], f8, name=f"vt{e2}", tag=f"vt{e2}")
             for e2 in range(NE2)]

    repeat = int(os.environ.get("ATTN_REPEAT", "1"))
    for rep in range(repeat):
        _emit_body(nc, tc, rep, locals())


def _emit_body(nc, tc, rep, env):
    import concourse.bass as bass
    from concourse import mybir
    from contextlib import ExitStack

    f32 = mybir.dt.float32
    f32r = mybir.dt.float32r
    f8 = mybir.dt.float8e4
    AF = mybir.ActivationFunctionType
    OP = mybir.AluOpType
    DR = mybir.MatmulPerfMode.DoubleRow

    (x_sb, q_sb, k_sb, vt_sb, wq_sb, wk_sb, wv_sb, wp_sb, bq_sb, bk_sb,
     bp_sb, nsc_sb, nbs_sb, bv_row, ind16_sb, indT_sb, oner_sb, ones8,
     expb_sb, osc_sb, out_d) = (
        env["x_sb"], env["q_sb"], env["k_sb"], env["vt_sb"], env["wq_sb"],
        env["wk_sb"], env["wv_sb"], env["wp_sb"], env["bq_sb"], env["bk_sb"],
        env["bp_sb"], env["nsc_sb"], env["nbs_sb"], env["bv_row"],
        env["ind16_sb"], env["indT_sb"], env["oner_sb"], env["ones8"],
        env["expb_sb"], env["osc_sb"], env["out_d"])

    h_pool = tc.alloc_tile_pool(name=f"hres{rep}", bufs=1, side="right")
    h_sb = [h_pool.tile([P, 2, HW], f8, name=f"h{cj}", tag=f"h{cj}")
            for cj in range(NCJ)]

    # ================ phase A: groupnorm ================
    with ExitStack() as s1:
        gn = s1.enter_context(tc.tile_pool(name="gn", bufs=2))
        gnp = s1.enter_context(tc.tile_pool(name="gnp", bufs=2, space="PSUM"))
        eps_t = gn.tile([8, 1], f32, name="eps_t", tag="eps", bufs=1)
        nc.vector.memset(eps_t, EPS)
        # fully per-chunk pipeline: groups are chunk-local, so each chunk's
        # aggregation + h-write runs as soon as ITS stats land, overlapping
        # later chunks' bn_stats instead of waiting on a cross-chunk barrier.
        for ci in range(NCH):
            stats = gn.tile([P, 8, 6], f32, name=f"st{ci}", tag="st")
            for s in range(8):
                nc.vector.bn_stats(out=stats[:, s, :],
                                   in_=x_sb[ci][:, s * 512:(s + 1) * 512])
            mv = gn.tile([P, 2], f32, name=f"mv{ci}", tag="mv")
            nc.vector.bn_aggr(out=mv, in_=stats)
            m2 = gn.tile([P, 2], f32, name=f"m2{ci}", tag="m2")
            nc.vector.tensor_copy(out=m2[:, 0:1], in_=mv[:, 0:1])
            msq = gn.tile([P, 1], f32, name=f"msq{ci}", tag="msq")
            nc.vector.tensor_mul(out=msq, in0=mv[:, 0:1], in1=mv[:, 0:1])
            nc.vector.tensor_add(out=m2[:, 1:2], in0=mv[:, 1:2], in1=msq)

            g_ps = gnp.tile([8, 2], f32, name=f"g{ci}", tag="gps")
            nc.tensor.matmul(g_ps, lhsT=ind16_sb, rhs=m2, start=True,
                             stop=True)
            g_sb = gn.tile([8, 2], f32, name=f"gsb{ci}", tag="gsb")
            nc.vector.tensor_copy(out=g_sb, in_=g_ps)
            msq2 = gn.tile([8, 1], f32, name=f"ms2{ci}", tag="msq2")
            nc.vector.tensor_mul(out=msq2, in0=g_sb[:, 0:1], in1=g_sb[:, 0:1])
            var_g = gn.tile([8, 1], f32, name=f"vg{ci}", tag="varg")
            nc.vector.tensor_sub(out=var_g, in0=g_sb[:, 1:2], in1=msq2)
            std_g = gn.tile([8, 1], f32, name=f"sg{ci}", tag="stdg")
            nc.scalar.activation(out=std_g, in_=var_g, func=AF.Sqrt,
                                 bias=eps_t, scale=1.0)
            rstd_g = gn.tile([8, 1], f32, name=f"rg{ci}", tag="rstdg")
            nc.vector.reciprocal(out=rstd_g, in_=std_g)
            rb2 = gn.tile([8, 2], f32, name=f"rb2{ci}", tag="rb2")
            nc.vector.tensor_copy(out=rb2[:, 0:1], in_=rstd_g)
            nc.vector.tensor_mul(out=rb2[:, 1:2], in0=g_sb[:, 0:1],
                                 in1=rstd_g)

            ab_ps = gnp.tile([P, 2], f32, name=f"ab{ci}", tag="gps")
            nc.tensor.matmul(ab_ps, lhsT=indT_sb, rhs=rb2, start=True,
                             stop=True)
            A_t = gn.tile([P, 1], f32, name=f"A{ci}", tag="A")
            nc.vector.tensor_mul(out=A_t, in0=ab_ps[:, 0:1], in1=nsc_sb[ci])
            t0 = gn.tile([P, 1], f32, name=f"t0{ci}", tag="t0")
            nc.vector.tensor_mul(out=t0, in0=ab_ps[:, 1:2], in1=nsc_sb[ci])
            B_t = gn.tile([P, 1], f32, name=f"B{ci}", tag="Bt")
            nc.vector.tensor_sub(out=B_t, in0=nbs_sb[ci], in1=t0)
            # h = A*x + B: DVE takes hf=0, Act takes hf=1 (act does
            # func(scale*in + bias) with per-partition scale/bias APs).
            nc.vector.tensor_scalar(
                out=h_sb[ci // 2][:, ci % 2, 0:2048],
                in0=x_sb[ci][:, 0:2048],
                scalar1=A_t, scalar2=B_t, op0=OP.mult, op1=OP.add)
            nc.scalar.activation(
                out=h_sb[ci // 2][:, ci % 2, 2048:4096],
                in_=x_sb[ci][:, 2048:4096],
                func=AF.Identity, bias=B_t, scale=A_t)

    # fold bp into the resident x: the phase-C tail then skips its Act bias
    # step (y = pj*rb + (x + bp)). Must come after the h-writes read x.
    # NOT on gpsimd: in-place [128,4096] tensor_scalar there takes 58us and
    # stalls concurrent DVE work.
    for oc in range(NCH):
        if oc % 2 == 0:
            nc.vector.tensor_scalar_add(out=x_sb[oc], in0=x_sb[oc],
                                        scalar1=bp_sb[oc])
        else:
            nc.scalar.activation(out=x_sb[oc], in_=x_sb[oc],
                                 func=AF.Identity, bias=bp_sb[oc], scale=1.0)

    # ================ phase B: q, k, vT projections (all SBUF-resident) ====
    with ExitStack() as s2:
        pp1 = s2.enter_context(tc.tile_pool(name="pp1", bufs=KNOBS["pp1"],
                                            space="PSUM"))
        # emitted in pixel-block order, k/q/v interleaved, so phase C's
        # early S/PV chains (which need only low pixel blocks of k/q/vt)
        # can overlap with the tail of phase B.
        for nb in range(NQB):
            for wsb, bsb, dst in ((wk_sb, bk_sb, k_sb), (wq_sb, bq_sb, q_sb)):
                for m in range(NCH):
                    ps = pp1.tile([P, QB], f32, name="qkps", tag="mm")
                    for cj in range(NCJ):
                        nc.tensor.matmul(
                            ps,
                            lhsT=wsb[cj][:, :, m * P:(m + 1) * P],
                            rhs=h_sb[cj][:, :, nb * QB:(nb + 1) * QB],
                            start=(cj == 0), stop=(cj == NCJ - 1),
                            perf_mode=DR)
                    if m % 2 == 0:
                        nc.scalar.activation(
                            out=dst[m // 2][:, m % 2, nb * QB:(nb + 1) * QB],
                            in_=ps, func=AF.Identity, bias=bsb[m], scale=1.0)
                    else:
                        nc.vector.tensor_scalar(
                            out=dst[m // 2][:, m % 2, nb * QB:(nb + 1) * QB],
                            in0=ps, scalar1=bsb[m], scalar2=None, op0=OP.add)
            # vT for the 4 key chunks of this pixel block. bv is seeded
            # into the PSUM chain by a broadcast f32r matmul (ones-row x
            # bv-row), so the conversion is a plain copy splittable
            # between DVE and Act.
            for e in range(nb * 4, nb * 4 + 4):
                ps = pp1.tile([P, C], f32, name="vps", tag="mm")
                nc.tensor.matmul(ps, lhsT=oner_sb, rhs=bv_row,
                                 start=True, stop=False)
                for cj in range(NCJ):
                    nc.tensor.matmul(
                        ps,
                        lhsT=h_sb[cj][:, :, e * P:(e + 1) * P],
                        rhs=wv_sb[cj],
                        start=False, stop=(cj == NCJ - 1),
                        perf_mode=DR)
                if e % 2 == 0:
                    nc.vector.tensor_copy(out=vt_sb[e // 2][:, e % 2, :],
                                          in_=ps)
                else:
                    nc.scalar.activation(out=vt_sb[e // 2][:, e % 2, :],
                                         in_=ps, func=AF.Copy,
                                         bias=0.0, scale=1.0)

    h_pool.release()

    # ================ phase C: attention + proj ================
    with ExitStack() as s3:
        ptp = s3.enter_context(tc.tile_pool(name="ptp", bufs=KNOBS["ptp"]))
        smp = s3.enter_context(tc.tile_pool(name="smp", bufs=2))
        osp = s3.enter_context(tc.tile_pool(name="osp", bufs=KNOBS["osp"]))
        fnp = s3.enter_context(tc.tile_pool(name="fnp", bufs=4))
        pv_ps_pool = s3.enter_context(tc.tile_pool(name="pvp", bufs=4,
                                                   space="PSUM"))
        s_ps_pool = s3.enter_context(tc.tile_pool(name="ssp",
                                                  bufs=KNOBS["ssp"],
                                                  space="PSUM"))
        dn_ps_pool = s3.enter_context(tc.tile_pool(name="dnp", bufs=1,
                                                   space="PSUM"))

        for qb in range(NQB):
            pvs = [pv_ps_pool.tile([P, QB], f32, name=f"pv{co}", tag="pv")
                   for co in range(NCH)]
            dn_ps = dn_ps_pool.tile([1, QB], f32, name="dn_ps", tag="dn")
            for e2 in range(NE2):
                pt = ptp.tile([P, 2, QB], f8, name="pt", tag="pt")
                for sub in range(2):
                    e = 2 * e2 + sub
                    s_ps = s_ps_pool.tile([P, QB], f32, name="s_ps", tag="s")
                    for cj in range(NCJ):
                        nc.tensor.matmul(
                            s_ps,
                            lhsT=k_sb[cj][:, :, e * P:(e + 1) * P],
                            rhs=q_sb[cj][:, :, qb * QB:(qb + 1) * QB],
                            start=(cj == 0), stop=(cj == NCJ - 1),
                            perf_mode=DR)
                    nc.scalar.activation(out=pt[:, sub, :], in_=s_ps,
                                         func=AF.Exp, bias=expb_sb,
                                         scale=float(SCALE))
                # denominator: fp8-ones DoubleRow over pt, accumulated on
                # the PE alongside the PV chain (one bank, 16 cheap MMs).
                nc.tensor.matmul(dn_ps, lhsT=ones8[:, :, 0:1], rhs=pt,
                                 start=(e2 == 0), stop=(e2 == NE2 - 1),
                                 perf_mode=DR)
                for co in range(NCH):
                    nc.tensor.matmul(
                        pvs[co],
                        lhsT=vt_sb[e2][:, :, co * P:(co + 1) * P],
                        rhs=pt,
                        start=(e2 == 0), stop=(e2 == NE2 - 1),
                        perf_mode=DR)
            # unnormalized PV out * 1/8 -> fp8 (IEEE e4m3 saturates at 240;
            # raw pvs reaches ~300). The 8x folds into rb via the dn copy
            # scale. Normalization commutes with the 1x1 conv: rb varies
            # only along queries, proj contracts channels. Emitted FIRST and
            # split DVE/Act so the pvs banks free quickly for the next qb.
            outp = [osp.tile([P, 2, QB], f8, name=f"op{cj}", tag="osb")
                    for cj in range(NCJ)]
            for m in range(NCH):
                if m % 2 == 0:
                    nc.vector.tensor_scalar(out=outp[m // 2][:, m % 2, :],
                                            in0=pvs[m], scalar1=osc_sb,
                                            scalar2=None, op0=OP.mult)
                else:
                    nc.scalar.activation(out=outp[m // 2][:, m % 2, :],
                                         in_=pvs[m], func=AF.Copy,
                                         bias=0.0, scale=0.125)
            # rb = 8/dn, built [1,512]-cheap: Act copy *0.125, DVE recip,
            # PE broadcast, DVE copy to SBUF.
            dn_sb = smp.tile([1, QB], f32, name="dn_sb", tag="dnsb", bufs=1)
            nc.scalar.activation(out=dn_sb, in_=dn_ps, func=AF.Copy,
                                 bias=0.0, scale=0.125)
            dn_r = smp.tile([1, QB], f32r, name="dn_r", tag="dnr", bufs=1)
            with nc.allow_low_precision(reason="f32r bit-identical to f32"):
                nc.vector.reciprocal(out=dn_r, in_=dn_sb)
            rb_ps = dn_ps_pool.tile([P, QB], f32, name="rb_ps", tag="dn")
            nc.tensor.matmul(rb_ps, lhsT=oner_sb, rhs=dn_r, start=True,
                             stop=True)
            rb_sb = smp.tile([P, QB], f32, name="rb_sb", tag="rbsb")
            nc.vector.tensor_copy(out=rb_sb, in_=rb_ps)
            for oc in range(NCH):
                pj_ps = pv_ps_pool.tile([P, QB], f32, name="pj_ps", tag="pv")
                for cj in range(NCJ):
                    nc.tensor.matmul(
                        pj_ps,
                        lhsT=wp_sb[cj][:, :, oc * P:(oc + 1) * P],
                        rhs=outp[cj],
                        start=(cj == 0), stop=(cj == NCJ - 1),
                        perf_mode=DR)
                t_n = fnp.tile([P, QB], f32, name="t_n", tag="tn",
                               bufs=KNOBS["tmp"])
                nc.vector.tensor_mul(out=t_n, in0=pj_ps, in1=rb_sb)
                fin = fnp.tile([P, QB], f32, name="fin", tag="fin",
                               bufs=KNOBS["fin"])
                eng = nc.gpsimd if oc % 2 == 0 else nc.vector
                eng.tensor_add(out=fin, in0=t_n,
                               in1=x_sb[oc][:, qb * QB:(qb + 1) * QB])
                deng = nc.sync if (qb + oc) % 2 == 0 else nc.scalar
                deng.dma_start(out=out_d[oc * P:(oc + 1) * P,
                                         qb * QB:(qb + 1) * QB], in_=fin)


def build_nc():
    import concourse.bacc as bacc
    import concourse.tile as tile
    from contextlib import ExitStack

    nc = bacc.Bacc("TRN2", target_bir_lowering=False, debug=False)
    with tile.TileContext(nc) as tc:
        with ExitStack() as ctx:
            _emit(nc, tc, ctx)
    nc.finalize()
    return nc


def _pack_wT(w):
    # w: [c_out, c_in] f32.  Return [cj*128+p, j, c_out] fp8 where
    # c_in = cj*256 + j*128 + p.
    wT = np.ascontiguousarray(np.asarray(w, np.float32).T)      # [c_in, c_out]
    wT = wT.reshape(NCJ, 2, P, C).transpose(0, 2, 1, 3)          # [cj, p, j, o]
    return np.ascontiguousarray(wT.reshape(NCJ * P, 2, C)).astype(
        ml_dtypes.float8_e4m3)


def _pack_consts(inputs):
    c = np.zeros((P, 32), np.float32)
    for m in range(NCH):
        c[:, CB_Q + m] = np.asarray(inputs["bq"], np.float32)[m * P:(m + 1) * P]
        c[:, CB_K + m] = np.asarray(inputs["bk"], np.float32)[m * P:(m + 1) * P]
        c[:, CB_P + m] = np.asarray(inputs["bp"], np.float32)[m * P:(m + 1) * P]
        c[:, CB_NS + m] = np.asarray(inputs["norm_scale"],
                                     np.float32)[m * P:(m + 1) * P]
        c[:, CB_NB + m] = np.asarray(inputs["norm_bias"],
                                     np.float32)[m * P:(m + 1) * P]
    for p in range(P):
        c[p, CB_I16 + p // 16] = 1.0 / 16.0
    return c


def make_in_maps(inputs):
    x = np.asarray(inputs["x"], np.float32)
    indT = np.zeros((8, P), np.float32)
    for p in range(P):
        indT[p // 16, p] = 1.0
    w8 = np.concatenate([_pack_wT(inputs["wq"]), _pack_wT(inputs["wk"]),
                         _pack_wT(inputs["wv"]), _pack_wT(inputs["wp"])],
                        axis=0)
    shared = {
        "w8": w8,
        "consts": _pack_consts(inputs),
        "indT": indT,
        "bv": np.asarray(inputs["bv"], np.float32),
    }
    return [dict(shared, x=np.ascontiguousarray(x[i].reshape(C, HW)))
            for i in range(B)]


def kernel(**inputs):
    from concourse.bass_utils import run_bass_kernel_spmd

    if "nc" not in _CACHE:
        _CACHE["nc"] = build_nc()
    nc = _CACHE["nc"]
    in_maps = make_in_maps(inputs)
    res = run_bass_kernel_spmd(nc, in_maps, list(range(B)))
    out = np.stack([res.results[i]["out"] for i in range(B)])
    return out.reshape(B, C, 64, 64)
